# revision 25
# baseline (speedup 1.0000x reference)
"""AdaPT int8-quantized 3x3 conv (exact) on 8 TRN2 NeuronCores.

Full inputs: x [32,8,384,384] f32, weight [8,8,3,3] f32, bias [8] f32.

Sharding: batch x height grid (2 batch-halves x 4 row-strips of 96 rows).
Each core gets x_core [16, 8, 98, 384] (rows pre-padded with the +-1 halo
on the host, zeros at image edges) and writes a padded output stream
out_core [16, 8, 96*386]; the host strips the 2 pad columns per row.
amax is per-tensor: local abs-max, then AllReduce(max) across cores.

Per-core plan (partition p = 8*img + ci everywhere):
  - load x in 8-row pieces spread over 3 DMA queues (SP/ACT/Pool) so
    transfers overlap; abs-max each piece as it lands (DVE XY-reduce and
    Pool XYZWC-reduce split the work); AllGather + local max across cores
  - quantize with the fp32 magic-number round (bit-exact RNE) into a
    PADDED bf16 stream qxh [128, 98, 386] (col 0/385 zero): all nine
    3x3 taps become stream offsets ky*386 + kx - 1 into this buffer
  - conv: per 512-wide PSUM bank, 9 accumulating matmuls with
    block-diagonal weights [128 K, 128 M] (K = 16 img x 8 ci,
    M = 16 img x 8 co) -- one matmul per tap covers all 16 images
  - evacuate psum*inv + bias into an 8-bank staging tile, one
    contiguous 2 MB DMA per group into the padded out stream
  - x is read 1.5x: the h0 half (rows 0:50) stays resident in xh f32
    for quantize; the h1 half (rows 50:98) is reloaded during quant-h0
"""

import numpy as np

N_CORES = 8
IMG = 16         # images per core
CI = 8
CO = 8
H = W = 384
WP = W + 2       # padded row width in the qx / out streams
ROWS = 96        # output rows per core strip
RH = ROWS + 2    # input rows incl halo
STREAM = ROWS * WP            # 37056, padded out stream length
S_BEG, S_END = 1, STREAM - 1  # real out positions [1, 37055)
NBANK = -(-(S_END - S_BEG) // 512)  # 73 (72 full + one 190)
GROUP = 8        # banks per staging tile / out DMA
MAGIC = 12582912.0  # 1.5 * 2**23, fp32 round-to-nearest-int trick
MAX_Q = 127.0

_cached = {}


def _build(n_cores=N_CORES, debug=False):
    from concourse import bacc, bass, tile, mybir, bass_isa

    f32 = mybir.dt.float32
    bf16 = mybir.dt.bfloat16

    nc = bacc.Bacc(
        "TRN2", target_bir_lowering=False, debug=debug, num_devices=n_cores
    )

    x_ext = nc.declare_dram_parameter("x", [IMG, CI, RH, W], f32, isOutput=False)
    w_ext = nc.declare_dram_parameter("weight", [CO, CI, 3, 3], f32, isOutput=False)
    b_ext = nc.declare_dram_parameter("bias", [CO], f32, isOutput=False)
    out_ext = nc.declare_dram_parameter("out", [IMG, CO, STREAM], f32, isOutput=True)

    # ---- static SBUF buffers (long-lived) ----
    sb = lambda name, shape, dt: nc.alloc_sbuf_tensor(name, list(shape), dt).ap()
    xh = sb("xh_s", [128, 50, W], f32)           # f32 staging (one half)
    qxh = sb("qxh_s", [128, RH, WP], bf16)       # padded quantized stream
    w24 = sb("w24_s", [24, 3 * CO], f32)         # w[co,ci,ky,kx] @ [8kx+ci, 8ky+co]
    w24q = sb("w24q_s", [24, 3, CO], bf16)
    qw_t = sb("qwt_s", [CI, 9, CO], bf16)        # [ci, g=3ky+kx, co]
    qwbig = sb("qwbig_s", [128, 9, 128], bf16)   # block-diag lhsT per tap
    aw = sb("aw_s", [24, 1], f32)
    aw_all = sb("awall_s", [24, 1], f32)
    sw = sb("sw_s", [24, 1], f32)
    bias_e = sb("biase_s", [128, 1], f32)
    axd = [sb(f"axd{j}_s", [128, 1], f32) for j in range(6)]  # DVE partials
    axp = [sb(f"axp{j}_s", [1, 1], f32) for j in range(7)]    # Pool partials
    ax0 = sb("ax0_s", [1, 1], f32)
    ax_all = sb("axall_s", [128, 1], f32)
    axg = sb("axg_s", [128, 1], f32)
    axg8 = sb("axg8_s", [1, n_cores], f32)
    axg0 = sb("axg0_s", [1, 1], f32)
    sx = sb("sx_s", [128, 1], f32)
    aw128 = sb("aw128_s", [128, 1], f32)
    inv = sb("inv_s", [128, 1], f32)

    qxh_f = qxh.rearrange("p r c -> p (r c)")
    qwbig_f = qwbig.rearrange("p g m -> p (g m)")

    # load piece: xh[:, r0:r0+n, :] = x_core[:, :, src0:src0+n, :]
    def load_piece(eng, r0, src0, n):
        eng.dma_start(
            out=xh[:, r0:r0 + n, :], in_=x_ext[:, :, src0:src0 + n, :]
        )

    with tile.TileContext(nc) as tc:
        with (
            tc.tile_pool(name="stage", bufs=2) as spool,
            tc.tile_pool(name="psum", bufs=8, space="PSUM") as pspool,
            tc.tile_pool(name="dram", bufs=1, space="DRAM") as dpool,
        ):
            # ---------------- x loads + amax, 3-queue parallel -------------
            # 8-row pieces; phase1 = h1 rows 50:98 (xh rows 0:48), phase2 =
            # h0 rows 0:50 (stays resident in xh for quantize).  Phase-2
            # piece k reuses phase-1 piece k's xh rows, so the WAR chain per
            # pair is load->amax->load->amax; small pieces keep it short.
            # Phase-2 loads go on a different queue than their phase-1 twin.
            qs = (nc.sync, nc.scalar, nc.gpsimd)
            nd = np_ = 0

            def amax_piece(j, r0, n):
                nonlocal nd, np_
                if j % 2 == 0:   # DVE
                    nc.vector.tensor_reduce(
                        axd[nd][:, :], xh[:, r0:r0 + n, :],
                        mybir.AxisListType.XY, mybir.AluOpType.max,
                        apply_absolute_value=True,
                    )
                    nd += 1
                else:            # Pool all-axis reduce -> [1,1]
                    nc.gpsimd.tensor_reduce(
                        axp[np_][:, :], xh[:, r0:r0 + n, :],
                        mybir.AxisListType.XYZWC, mybir.AluOpType.max,
                        apply_absolute_value=True,
                    )
                    np_ += 1

            for j in range(6):                    # phase 1: h1 rows
                r0 = 8 * j
                load_piece(qs[j % 3], r0, 50 + r0, 8)
                amax_piece(j, r0, 8)
            # rows 48:50 overlap no phase-1 piece: load early, off-chain
            load_piece(nc.scalar, 48, 48, 2)
            amax_piece(1, 48, 2)                  # Pool
            # phase-2 loads stay off the gpsimd queue: SWDGE desc-gen runs on
            # the Pool engine and head-of-line blocks the Pool amax reduces
            for j in range(6):                    # phase 2: h0 rows
                r0 = 8 * j
                load_piece(qs[j % 2], r0, r0, 8)
                amax_piece(j, r0, 8)

            # ---------------- combine amax partials ----------------
            # sequential folds: each runs as soon as its piece lands, so
            # only the final fold trails the last amax
            for k in range(1, nd):
                nc.vector.tensor_tensor(
                    out=axd[0][:, :], in0=axd[0][:, :], in1=axd[k][:, :],
                    op=mybir.AluOpType.max,
                )
            for k in range(1, np_):
                nc.vector.tensor_tensor(
                    out=axp[0][:, :], in0=axp[0][:, :], in1=axp[k][:, :],
                    op=mybir.AluOpType.max,
                )
            nc.gpsimd.partition_all_reduce(
                ax_all[:, :], axd[0][:, :], channels=128,
                reduce_op=bass_isa.ReduceOp.max,
            )
            nc.vector.tensor_tensor(
                out=ax0[:, :], in0=ax_all[0:1, :], in1=axp[0][:, :],
                op=mybir.AluOpType.max,
            )
            # ------------- amax exchange: AllGather + local max -------------
            cc_in = dpool.tile([1, 1], f32)
            cc_out = dpool.tile([1, n_cores], f32)
            nc.sync.dma_start(out=cc_in[:, :], in_=ax0[:, :])
            nc.gpsimd.collective_compute(
                "AllGather",
                mybir.AluOpType.bypass,
                replica_groups=[list(range(n_cores))],
                ins=[cc_in.opt()],
                outs=[cc_out.opt()],
            )
            nc.sync.dma_start(out=axg8[:, :], in_=cc_out[:, :])

            # ---------------- weight prep (scalar queue, off load path) ----
            with nc.allow_non_contiguous_dma(reason="one-time 576-elem w load"):
                for ky in range(3):
                    for kx in range(3):
                        dst = w24[8 * kx:8 * kx + 8, CO * ky:CO * ky + CO]
                        src = w_ext[:, :, ky, kx].rearrange("co ci -> ci co")
                        nc.scalar.dma_start(out=dst, in_=src)

            nc.vector.tensor_reduce(
                aw[:, :], w24[:, :], mybir.AxisListType.X, mybir.AluOpType.max,
                apply_absolute_value=True,
            )
            nc.gpsimd.partition_all_reduce(
                aw_all[:, :], aw[:, :], channels=24, reduce_op=bass_isa.ReduceOp.max
            )
            nc.vector.reciprocal(sw[:, :], aw_all[:, :])
            nc.vector.tensor_scalar(
                out=sw[:, :], in0=sw[:, :], scalar1=MAX_Q, scalar2=None,
                op0=mybir.AluOpType.mult,
            )
            # quantize weights: round(w * sw) via magic, to bf16
            nc.vector.tensor_scalar(
                out=w24[:, :], in0=w24[:, :], scalar1=sw[:, :], scalar2=MAGIC,
                op0=mybir.AluOpType.mult, op1=mybir.AluOpType.add,
            )
            nc.scalar.activation(
                w24q.rearrange("p a b -> p (a b)"), w24[:, :],
                mybir.ActivationFunctionType.Copy, bias=-MAGIC, scale=1.0,
            )
            # qw_t[ci, 3ky+kx, co] = w24q[8kx+ci, ky, co]
            with nc.allow_non_contiguous_dma(reason="one-time w rearrange"):
                for kx in range(3):
                    nc.scalar.dma_start(
                        out=qw_t[:, kx::3, :],
                        in_=w24q[8 * kx:8 * kx + 8, :, :],
                    )
                nc.vector.memset(qwbig[:, :, :], 0.0)
                for i in range(IMG):
                    nc.scalar.dma_start(
                        out=qwbig[8 * i:8 * i + 8, :, 8 * i:8 * i + 8],
                        in_=qw_t[:, :, :],
                    )

            # bias vector on evac partitions p = 8*img + co: log-doubling
            nc.scalar.dma_start(out=bias_e[0:CO, :], in_=b_ext[:])
            for m in (8, 16, 32, 64):
                nc.scalar.dma_start(out=bias_e[m:2 * m, :], in_=bias_e[0:m, :])

            # zero the pad columns of the qx stream (quantize never writes them)
            nc.vector.memset(qxh[:, :, 0:1], 0.0)
            nc.vector.memset(qxh[:, :, WP - 1:WP], 0.0)

            nc.vector.tensor_reduce(
                axg0[:, :], axg8[:, :], mybir.AxisListType.X,
                mybir.AluOpType.max,
            )
            nc.gpsimd.partition_broadcast(axg[:, :], axg0[:, :])

            # sx = 127/axg  (per-partition, all equal)
            nc.vector.reciprocal(sx[:, :], axg[:, :])
            nc.vector.tensor_scalar(
                out=sx[:, :], in0=sx[:, :], scalar1=MAX_Q, scalar2=None,
                op0=mybir.AluOpType.mult,
            )
            # inv = axg * aw / 127^2
            nc.gpsimd.partition_broadcast(aw128[:, :], aw_all[0:1, :])
            nc.vector.tensor_tensor(
                out=inv[:, :], in0=axg[:, :], in1=aw128[:, :],
                op=mybir.AluOpType.mult,
            )
            nc.vector.tensor_scalar(
                out=inv[:, :], in0=inv[:, :], scalar1=1.0 / (MAX_Q * MAX_Q),
                scalar2=None, op0=mybir.AluOpType.mult,
            )

            # ---------------- quantize ----------------
            # DVE: xh = xh*sx + MAGIC (in place); ACT: qxh = Copy(xh - MAGIC)
            def quant(r0, n, q0):
                nc.vector.tensor_scalar(
                    out=xh[:, r0:r0 + n, :], in0=xh[:, r0:r0 + n, :],
                    scalar1=sx[:, :], scalar2=MAGIC,
                    op0=mybir.AluOpType.mult, op1=mybir.AluOpType.add,
                )
                nc.scalar.activation(
                    qxh[:, q0 + r0:q0 + r0 + n, 1:W + 1], xh[:, r0:r0 + n, :],
                    mybir.ActivationFunctionType.Copy, bias=-MAGIC, scale=1.0,
                )

            for (r0, n) in ((0, 4), (4, 20), (24, 26)):   # h0 resident in xh
                quant(r0, n, 0)
            # reload h1 on two queues, quantize as pieces land
            load_piece(nc.sync, 0, 50, 24)
            quant(0, 24, 50)
            load_piece(nc.gpsimd, 24, 74, 24)
            quant(24, 24, 50)

            # ---------------- conv: 9 taps x 73 banks ----------------
            evac_cycle = (nc.scalar, nc.vector)
            out_q = (nc.sync, nc.gpsimd)
            bank = 0
            g_out = 0
            group_sizes = [GROUP] * 8 + [5, 3, 1]
            assert sum(group_sizes) == NBANK
            while bank < NBANK:
                nb = group_sizes[g_out]
                glen = sum(
                    min(512, S_END - (S_BEG + 512 * (bank + k))) for k in range(nb)
                )
                st = spool.tile([128, GROUP * 512], f32, tag="st")
                for k in range(nb):
                    b = bank + k
                    s_a = S_BEG + 512 * b
                    N = min(512, S_END - s_a)
                    ps = pspool.tile([128, 512], f32, tag="ps")
                    for g9 in range(9):
                        ky, kx = divmod(g9, 3)
                        off = s_a + ky * WP + kx - 1
                        nc.tensor.matmul(
                            ps[:, 0:N],
                            qwbig_f[:, 128 * g9:128 * g9 + 128],
                            qxh_f[:, off:off + N],
                            start=(g9 == 0),
                            stop=(g9 == 8),
                        )
                    eng = evac_cycle[b % 2]
                    if eng is nc.scalar:
                        nc.scalar.activation(
                            st[:, 512 * k:512 * k + N], ps[:, 0:N],
                            mybir.ActivationFunctionType.Identity,
                            bias=bias_e[:, :], scale=inv[:, :],
                        )
                    else:
                        eng.tensor_scalar(
                            out=st[:, 512 * k:512 * k + N], in0=ps[:, 0:N],
                            scalar1=inv[:, :], scalar2=bias_e[:, :],
                            op0=mybir.AluOpType.mult, op1=mybir.AluOpType.add,
                        )
                s_g = S_BEG + 512 * bank
                out_q[g_out % 2].dma_start(
                    out=out_ext[:, :, s_g:s_g + glen], in_=st[:, 0:glen]
                )
                bank += nb
                g_out += 1

    nc.compile()
    return nc


def _get_nc():
    if "nc" not in _cached:
        _cached["nc"] = _build()
    return _cached["nc"]


def make_core_inputs(x, weight, bias):
    """Shard full inputs into per-core input maps (host side)."""
    x = np.ascontiguousarray(x, dtype=np.float32)
    weight = np.ascontiguousarray(weight, dtype=np.float32)
    bias = np.ascontiguousarray(bias, dtype=np.float32)
    in_maps = []
    for core in range(N_CORES):
        b, h = divmod(core, 4)
        xc = np.zeros((IMG, CI, RH, W), dtype=np.float32)
        lo = 96 * h - 1
        src_lo, src_hi = max(lo, 0), min(lo + RH, H)
        xc[:, :, src_lo - lo:src_hi - lo, :] = (
            x[IMG * b:IMG * b + IMG, :, src_lo:src_hi, :]
        )
        in_maps.append({"x": xc, "weight": weight, "bias": bias})
    return in_maps


def assemble_output(results):
    """Gather per-core padded streams into the full output."""
    out = np.empty((2 * IMG, CO, H, W), dtype=np.float32)
    for core in range(N_CORES):
        b, h = divmod(core, 4)
        strip = results[core]["out"].reshape(IMG, CO, ROWS, WP)[:, :, :, 1:W + 1]
        out[IMG * b:IMG * b + IMG, :, 96 * h:96 * h + ROWS, :] = strip
    return out


def kernel(x, weight, bias):
    from concourse.bass_utils import run_bass_kernel_spmd

    nc = _get_nc()
    in_maps = make_core_inputs(x, weight, bias)
    res = run_bass_kernel_spmd(nc, in_maps, core_ids=list(range(N_CORES)))
    return assemble_output(res.results)
